# revision 5
# baseline (speedup 1.0000x reference)
"""Trainium2 Bass kernel for MoGNN forward (global mean-pool + linear).

pooled[g] = mean over nodes n with batch[n] == g of x[n]   # [1024, 512]
out = pooled @ W.T + b                                     # [1024, 7]

Sharded by graphs: core k owns graphs [128k, 128k+128) and their contiguous
node rows (batch ids sorted). No collectives; each core writes its [128, 7]
slice and the host concatenates. Measured ~38us vs the 51us fp16 baseline.

  - x ships as fp8 e4m3 (1 byte/elt; stream measured at the ~358 GB/s/core
    HBM roofline). Host quantization uses per-(graph,feature) error
    diffusion: the rounding carry propagates down each graph's node run so
    the per-graph SUM of shipped values matches fp32 to ~1 ulp. End-to-end
    rel err 2.3e-3 (gate 2e-2).
  - DoubleRow fp8 matmuls: one-hot stationary [128, 2, 128], x moving
    [128, 2, 512] -> 256 nodes contracted per PE instruction.
  - the one-hot is built from batch ids on the otherwise-idle DVE
    (is_equal with step-0 broadcast APs, fp16 in -> fp8 out), in small
    front-loaded pieces that stay ahead of the x stream.
  - x DMA alternates the two HWDGE rings (sync/scalar) per 1MB chunk;
    head ramp + tail taper; bufs=6 so issues never stall on pool frees.
  - HAM clock-gate management: DMA-independent 1-col dummy matmuls warm
    the PE right after the engine preamble, and inter-chunk keepalive
    dummies hold it warm through DMA-paced idle gaps (without them the
    gate re-throttles mid-stream and matmuls run 1.67x slower).
  - epilogue: per-128-col scale by 1/count (fp32 PSUM -> fp16), 4 PE
    transposes to feat-major, 4-matmul classifier with pooled.T stationary
    and W moving (N=7), bias via a partition-replicated tile.
"""

import numpy as np

NCORES = 8
G = 1024            # total graphs
GPC = G // NCORES   # graphs per core = 128
F = 512             # feature dim
P = 128             # partition / node-tile size
CHUNK = 16          # node tiles per DMA chunk (1 MB fp8 transfers)
OHCHUNK = 8         # node tiles per one-hot build op (DVE/GpSimd alternate)
WARMUP = 28         # dummy PE transposes to trip the HAM clock-gate early

_compiled_cache = {}


def _chunk_plan(ntiles):
    """Even-length chunks: small head so the PE starts early, CHUNK steady
    state, telescoping taper so the PE's chunk-granularity lag shrinks to
    ~2 tiles by the final transfer."""
    head = [2, 6]
    tail = [8, 4]
    chunks = []
    t0 = 0
    main_end = max(ntiles - sum(tail), 0)
    for ramp in head:
        if t0 < main_end:
            clen = min(ramp, main_end - t0)
            chunks.append((t0, clen))
            t0 += clen
    while t0 < main_end:
        clen = min(CHUNK, main_end - t0)
        chunks.append((t0, clen))
        t0 += clen
    while t0 < ntiles:
        clen = min(tail.pop(0) if tail else CHUNK, ntiles - t0)
        chunks.append((t0, clen))
        t0 += clen
    assert sum(c for _, c in chunks) == ntiles
    assert all(c0 % 2 == 0 and clen % 2 == 0 for c0, clen in chunks)
    return chunks


def _build(ntiles, num_devices=NCORES):
    """Build + compile the per-core Bass kernel for `ntiles` node tiles
    (ntiles even)."""
    from concourse import bacc, tile, mybir

    f32 = mybir.dt.float32
    f16 = mybir.dt.float16
    f8 = mybir.dt.float8e4
    u8 = mybir.dt.uint8
    eq = mybir.AluOpType.is_equal
    mult = mybir.AluOpType.mult
    add = mybir.AluOpType.add
    DR = mybir.MatmulPerfMode.DoubleRow

    assert ntiles % 2 == 0
    nrows = ntiles * P
    chunks = _chunk_plan(ntiles)

    nc = bacc.Bacc(
        "TRN2",
        target_bir_lowering=False,
        debug=False,
        num_devices=num_devices,
    )

    # x shard chunk-contiguous, partition-major inside each chunk:
    # block[p, t, m] = x[(c0+t)*128+p, m] per chunk
    x_d = nc.dram_tensor("xs", [nrows * F], u8, kind="ExternalInput")
    # cp16 = [bl (ntiles) | iota (GPC) | ident (P) | wtr (28)]
    cp16_d = nc.dram_tensor(
        "cp16", [P, ntiles + GPC + P + 28], f16, kind="ExternalInput"
    )
    # cp32 = [b replicated (7) | icnt per-partition (1)]
    cp32_d = nc.dram_tensor("cp32", [P, 8], f32, kind="ExternalInput")
    out_d = nc.dram_tensor("out", [GPC, 7], f32, kind="ExternalOutput")

    with tile.TileContext(nc) as tc:
        with (
            tc.tile_pool(name="const", bufs=1) as constp,
            tc.tile_pool(name="oh", bufs=1) as ohp,
            tc.tile_pool(name="xin", bufs=6) as xp,
            tc.tile_pool(name="acc", bufs=1, space="PSUM") as accp,
            tc.tile_pool(name="dum", bufs=1, space="PSUM") as dump,
            tc.tile_pool(name="tps", bufs=2, space="PSUM") as tpsp,
            tc.tile_pool(name="outp", bufs=1, space="PSUM") as outpp,
            tc.tile_pool(name="sb", bufs=1) as sbp,
        ):
            # HAM warmup with no DMA dependency: memset a tiny SBUF tile on
            # the DVE, then feed the PE 1-col dummy matmuls starting right at
            # preamble-end (~5.5us) so the clock gate opens (~3.4us of
            # sustained activity) before the first real matmul ever issues
            warm_sb = constp.tile([P, 4], f16)
            nc.vector.memset(warm_sb[:], 1.0)
            dum_ps = dump.tile([1, 1], f32)
            for _ in range(2 * WARMUP):
                nc.tensor.matmul(
                    dum_ps[:],
                    warm_sb[:, 0:1],
                    warm_sb[:, 0:1],
                    start=True,
                    stop=True,
                )

            # cp16 rides first on the sync ring so the one-hot build can
            # start as early as possible; cp32 on the scalar ring
            cp16_t = constp.tile([P, ntiles + GPC + P + 28], f16)
            nc.sync.dma_start(cp16_t[:], cp16_d.ap())
            cp32_t = constp.tile([P, 8], f32)
            nc.scalar.dma_start(cp32_t[:], cp32_d.ap())
            bl_t = cp16_t[:, 0:ntiles]
            iota_t = cp16_t[:, ntiles : ntiles + GPC]
            ident_t = cp16_t[:, ntiles + GPC : ntiles + GPC + P]
            wtr_t = cp16_t[:, ntiles + GPC + P : ntiles + GPC + P + 28]
            brep_t = cp32_t[:, 0:7]
            icnt_t = cp32_t[:, 7:8]



            # one-hot, built piecewise on DVE: ohm[p, t, g] = (bl[p,t] == g).
            # Small pieces keep the build ahead of the x stream per-tile.
            ohm = ohp.tile([P, ntiles, GPC], f8)
            iota_rep = iota_t.rearrange("p (a g) -> p a g", a=1)
            s0 = 0
            while s0 < ntiles:
                sl = min(4 if s0 < 8 else OHCHUNK, ntiles - s0)
                nc.vector.tensor_tensor(
                    ohm[:, s0 : s0 + sl, :],
                    iota_rep.broadcast_to([P, sl, GPC]),
                    bl_t[:, s0 : s0 + sl]
                    .rearrange("p (n a) -> p n a", a=1)
                    .broadcast_to([P, sl, GPC]),
                    op=eq,
                )
                s0 += sl

            acc = accp.tile([GPC, F], f32)
            # keepalive: 1-col matmuls between chunks run in the PE's
            # DMA-paced idle gaps so the HAM activity monitor never
            # re-throttles the clock mid-stream (seen as K=8->4 at ~19us
            # with 1.67x slower matmuls until ~26us)
            x_flat = x_d.ap()

            for ci, (c0, clen) in enumerate(chunks):
                xt = xp.tile([P, CHUNK, F], u8, tag="xt")
                chunk_ap = x_flat[c0 * P * F : (c0 + clen) * P * F].rearrange(
                    "(p t m) -> p t m", p=P, m=F
                )
                # alternate the two HWDGE rings so one ring's completion
                # bubbles overlap the other's data; scalar takes chunk 0
                # since sync is busy shipping cp16 first
                dma_eng = nc.scalar if ci % 2 == 0 else nc.sync
                dma_eng.dma_start(xt[:, :clen, :], chunk_ap)
                for u in range(clen // 2):
                    td = c0 + 2 * u
                    nc.tensor.matmul(
                        acc[:],
                        ohm[:, td : td + 2, :],
                        xt[:, 2 * u : 2 * u + 2, :].bitcast(f8),
                        start=(td == 0),
                        stop=(td == ntiles - 2),
                        perf_mode=DR,
                    )
                if ci < len(chunks) - 2:
                    for _ in range(8):
                        nc.tensor.matmul(
                            dum_ps[:],
                            ident_t[:, 0:1],
                            ident_t[:, 0:1],
                            start=True,
                            stop=True,
                        )

            # pooled = acc * (1/count), sliced so the fp16 transposes pipeline
            # behind the scale copies; classifier with pooled.T stationary
            pooled = sbp.tile([GPC, F], f16)
            ptall = sbp.tile([P, 4, P], f16)
            for j in range(4):
                sl = slice(j * P, (j + 1) * P)
                nc.vector.tensor_scalar(
                    pooled[:, sl], acc[:, sl], icnt_t, None, op0=mult
                )
                tp = tpsp.tile([P, P], f16)
                nc.tensor.transpose(tp[:], pooled[:, sl], ident_t)
                nc.vector.tensor_copy(ptall[:, j, :], tp[:])

            out_ps = outpp.tile([GPC, 7], f32)
            for j in range(4):
                nc.tensor.matmul(
                    out_ps[:],
                    ptall[:, j, :],
                    wtr_t[:, j * 7 : (j + 1) * 7],
                    start=(j == 0),
                    stop=(j == 3),
                )

            out_sb = sbp.tile([GPC, 7], f32)
            nc.vector.tensor_tensor(out_sb[:], out_ps[:], brep_t, op=add)
            nc.sync.dma_start(out_d.ap(), out_sb[:])

    nc.compile()
    return nc


def _get_compiled(ntiles):
    if ntiles not in _compiled_cache:
        _compiled_cache[ntiles] = _build(ntiles)
    return _compiled_cache[ntiles]


def _quantize_diffused(x, batch, counts):
    """e4m3 quantization with per-(graph,feature) error diffusion along each
    graph's node run. Returns uint8 view [N, F] of the e4m3 bytes."""
    import ml_dtypes

    f8 = ml_dtypes.float8_e4m3
    N = x.shape[0]
    starts = np.searchsorted(batch, np.arange(G))
    xq = np.empty((N, F), dtype=np.uint8)
    carry = np.zeros((G, F), dtype=np.float32)
    maxc = int(counts.max())
    for i in range(maxc):
        sel = counts > i
        rows = starts[sel] + i
        v = x[rows] + carry[sel]
        q = v.astype(f8)
        xq[rows] = q.view(np.uint8)
        carry[sel] = v - q.astype(np.float32)
    return xq


def _prep_in_maps(xq, batch, W, b, ntiles, bounds, inv_counts):
    cap = ntiles * P
    chunk_plan = _chunk_plan(ntiles)
    iota = np.tile(np.arange(GPC, dtype=np.float16)[None, :], (P, 1))
    wtr = np.ascontiguousarray(
        W.T.reshape(4, P, 7).transpose(1, 0, 2).reshape(P, 28)
    ).astype(np.float16)
    cp32_base = np.zeros((P, 8), dtype=np.float32)
    cp32_base[:, 0:7] = b.astype(np.float32)[None, :]

    in_maps = []
    for k in range(NCORES):
        lo, hi = int(bounds[k]), int(bounds[k + 1])
        n = hi - lo
        xs = np.zeros((cap, F), dtype=np.uint8)
        xs[:n] = xq[lo:hi]
        xs = xs.reshape(ntiles, P, F)
        parts = [
            np.ascontiguousarray(xs[c0 : c0 + clen].transpose(1, 0, 2)).reshape(-1)
            for c0, clen in chunk_plan
        ]
        xs = np.concatenate(parts)
        blv = np.full((cap,), -1.0, dtype=np.float16)
        blv[:n] = (batch[lo:hi] - GPC * k).astype(np.float16)
        cp16 = np.empty((P, ntiles + GPC + P + 28), dtype=np.float16)
        cp16[:, 0:ntiles] = blv.reshape(ntiles, P).T
        cp16[:, ntiles : ntiles + GPC] = iota
        cp16[:, ntiles + GPC : ntiles + GPC + P] = np.eye(P, dtype=np.float16)
        cp16[:, ntiles + GPC + P :] = wtr
        cp32 = cp32_base.copy()
        cp32[:, 7] = inv_counts[GPC * k : GPC * (k + 1)]
        in_maps.append({"xs": xs, "cp16": cp16, "cp32": cp32})
    return in_maps


_last_result = None  # test harness can read exec_time_ns / trace from here


def kernel(x, edge_index, edge_attr, batch_size, W, b):
    from concourse import bass_utils

    global _last_result

    x = np.asarray(x, dtype=np.float32)
    batch = np.asarray(batch_size).astype(np.int64)
    W = np.asarray(W, dtype=np.float32)
    b = np.asarray(b, dtype=np.float32)

    if batch.size > 1 and np.any(np.diff(batch) < 0):
        order = np.argsort(batch, kind="stable")
        batch = batch[order]
        x = x[order]

    counts = np.bincount(batch, minlength=G)
    inv_counts = (1.0 / np.maximum(counts, 1)).astype(np.float32)
    bounds = np.searchsorted(batch, np.arange(0, G + 1, GPC))
    max_rows = int(np.diff(bounds).max())
    ntiles = max(-(-max_rows // P), 2)
    ntiles += ntiles % 2  # DoubleRow pairs tiles

    xq = _quantize_diffused(x, batch, counts)
    nc = _get_compiled(ntiles)
    in_maps = _prep_in_maps(xq, batch, W, b, ntiles, bounds, inv_counts)

    res = bass_utils.run_bass_kernel_spmd(
        nc, in_maps, core_ids=list(range(NCORES))
    )
    _last_result = res

    out = np.concatenate(
        [np.asarray(res.results[k]["out"]) for k in range(NCORES)], axis=0
    )
    return np.ascontiguousarray(out.astype(np.float32))


def _selftest_sim():
    """Scaled-down CoreSim validation: tiny node count, synthetic graphs."""
    import ml_dtypes
    from concourse.bass_interp import CoreSim

    rng = np.random.default_rng(0)
    ntiles = 6
    n = ntiles * P - 37
    batch_local = np.sort(rng.integers(0, GPC, size=n))
    xsmall = rng.standard_normal((n, F), dtype=np.float32)
    Wt = rng.standard_normal((7, F), dtype=np.float32) * 0.02
    bt = rng.standard_normal(7).astype(np.float32) * 0.02

    counts = np.bincount(batch_local, minlength=GPC)
    inv_counts = (1.0 / np.maximum(counts, 1)).astype(np.float32)

    f8 = ml_dtypes.float8_e4m3
    xq = xsmall.astype(f8)
    xqb = xq.view(np.uint8)

    nc = _build(ntiles, num_devices=1)

    cap = ntiles * P
    xs = np.zeros((cap, F), dtype=np.uint8)
    xs[:n] = xqb
    xs = xs.reshape(ntiles, P, F)
    parts = [
        np.ascontiguousarray(xs[c0 : c0 + clen].transpose(1, 0, 2)).reshape(-1)
        for c0, clen in _chunk_plan(ntiles)
    ]
    xs = np.concatenate(parts)
    iota = np.tile(np.arange(GPC, dtype=np.float16)[None, :], (P, 1))
    wtr = np.ascontiguousarray(
        Wt.T.reshape(4, P, 7).transpose(1, 0, 2).reshape(P, 28)
    ).astype(np.float16)
    blv = np.full((cap,), -1.0, dtype=np.float16)
    blv[:n] = batch_local.astype(np.float16)
    cp16 = np.empty((P, ntiles + GPC + P + 28), dtype=np.float16)
    cp16[:, 0:ntiles] = blv.reshape(ntiles, P).T
    cp16[:, ntiles : ntiles + GPC] = iota
    cp16[:, ntiles + GPC : ntiles + GPC + P] = np.eye(P, dtype=np.float16)
    cp16[:, ntiles + GPC + P :] = wtr
    cp32 = np.zeros((P, 8), dtype=np.float32)
    cp32[:, 0:7] = bt[None, :]
    cp32[:, 7] = inv_counts

    sim = CoreSim(nc)
    sim.tensor("xs")[:] = xs
    sim.tensor("cp16")[:] = cp16
    sim.tensor("cp32")[:] = cp32
    sim.simulate()
    got = np.array(sim.tensor("out"))

    sums = np.zeros((GPC, F), dtype=np.float64)
    np.add.at(sums, batch_local, xq.astype(np.float64))
    pooled = (sums / np.maximum(counts, 1)[:, None]).astype(np.float16)
    want = pooled.astype(np.float64) @ Wt.astype(np.float16).astype(
        np.float64
    ).T + bt
    rel = np.linalg.norm(got - want) / np.linalg.norm(want)
    print(f"sim rel err vs oracle: {rel:.3e}")
    assert rel < 2e-3, rel
    print("SIM PASS")


if __name__ == "__main__":
    import sys

    if len(sys.argv) > 1 and sys.argv[1] == "sim":
        _selftest_sim()
